# revision 21
# baseline (speedup 1.0000x reference)
"""Trainium2 Bass kernel for a 4-layer transformer encoder (B=8,S=1024,D=512,H=8,FF=2048).

Sharding: data-parallel over batch -- one batch element per NeuronCore (8 cores).
Each core runs the full 4-layer encoder on its (S, D) slice with replicated weights.

On-device design (per core, feature-major activations xT [D, S]):
 - all matmuls in fp32r (11-bit mantissa, 1 cyc/row); weights pre-rounded on host
 - q,k kept feature-major; v token-major with an extra ones column per head so the
   attention@V matmul also produces the softmax denominator (row 64 of PSUM out)
 - softmax without max-subtraction (scores are O(1)); length mask folded into the
   Exp activation's per-partition bias; masked keys give exp()=0 exactly
 - layernorm in feature-major: partition-dim sums via ones[128,128] matmuls,
   unbiased std + eps per the torch-style reference
SBUF slot sharing: z1 reuses qT slots, y1 reuses kT slots, next-layer x reuses x
slots (all phase-disjoint). PSUM: one 4-bank "A" slot (manual halves) + 4x 1-bank
"B" slots + 1 2-bank attention accumulator = 8 banks.
"""
import math
import numpy as np
from contextlib import ExitStack

import concourse.bass as bass
import concourse.tile as tile
from concourse import bacc, mybir
from concourse.bass_utils import run_bass_kernel_spmd

B, S, D, H, FF, L = 8, 1024, 512, 8, 2048, 4
DH = D // H
EPS = 1e-6
NCORES = 8
FP32 = mybir.dt.float32
FP32R = mybir.dt.float32r
AF = mybir.ActivationFunctionType
OP = mybir.AluOpType

DC = D // 128      # 4 feature chunks
SH = S // 512      # 2 sequence halves
ST = S // 128      # 8 sequence chunks
FC = FF // 128     # 16 ff chunks
NEG = -30000.0     # additive mask; exp(x + NEG) underflows to exactly 0


def round_fp32r(a: np.ndarray) -> np.ndarray:
    """Round-to-nearest-even fp32 -> fp32r (11-bit mantissa). Matches HW."""
    bits = np.ascontiguousarray(a, dtype=np.float32).view(np.uint32)
    lsb = (bits >> 12) & 1
    return ((bits + 0x7FF + lsb) & np.uint32(0xFFFFF000)).view(np.float32)


def _pe_table() -> np.ndarray:
    pos = np.arange(S, dtype=np.float32)[:, None]
    div = np.exp(np.arange(0, D, 2, dtype=np.float32) * (-math.log(10000.0) / D))
    pe = np.zeros((S, D), dtype=np.float32)
    pe[:, 0::2] = np.sin(pos * div)
    pe[:, 1::2] = np.cos(pos * div)
    return pe


def build_nc(n_layers: int = L, debug_phase: str = ""):
    nc = bacc.Bacc("TRN2", target_bir_lowering=False, debug=False,
                   num_devices=NCORES)

    dt = nc.dram_tensor
    xT = dt("xT", [D, S], FP32, kind="ExternalInput").ap()
    peT = dt("peT", [D, S], FP32, kind="ExternalInput").ap()
    maskB = dt("maskB", [128, ST], FP32, kind="ExternalInput").ap()
    Wq = dt("Wq", [L, D, D], FP32R, kind="ExternalInput").ap()
    Wk = dt("Wk", [L, D, D], FP32R, kind="ExternalInput").ap()
    Wv = dt("Wv", [L, D, D], FP32R, kind="ExternalInput").ap()
    Wo = dt("Wo", [L, D, D], FP32R, kind="ExternalInput").ap()
    W1 = dt("W1", [L, D, FF], FP32R, kind="ExternalInput").ap()
    W2 = dt("W2", [L, FF, D], FP32R, kind="ExternalInput").ap()
    ball = dt("ball", [L, 128, 8 * DC + FC], FP32, kind="ExternalInput").ap()
    bv = dt("bv", [L, D], FP32R, kind="ExternalInput").ap()
    outT = dt("outT", [D, S], FP32, kind="ExternalOutput").ap()

    with tile.TileContext(nc) as tc, ExitStack() as ctx:
        ec = ctx.enter_context
        const = ec(tc.tile_pool(name="const", bufs=1))
        big = ec(tc.tile_pool(name="big", bufs=1))       # persistent activations
        tmp1k = ec(tc.tile_pool(name="tmp1k", bufs=3))   # [128,1024] transients
        tmp5 = ec(tc.tile_pool(name="tmp5", bufs=2))     # [128,512] transients
        stat = ec(tc.tile_pool(name="stat", bufs=1))     # LN stats tiles
        wpool = ec(tc.tile_pool(name="w", bufs=2))       # QKVO weight row tiles
        wstrm = ec(tc.tile_pool(name="wstrm", bufs=4))   # streamed W1/W2 tiles
        bias = ec(tc.tile_pool(name="bias", bufs=2))
        expp = ec(tc.tile_pool(name="exp", bufs=4))
        psA = ec(tc.tile_pool(name="psA", bufs=1, space="PSUM"))  # 4-bank slot
        psB = ec(tc.tile_pool(name="psB", bufs=4, space="PSUM"))  # 1-bank slots

        # ---- constants ----
        mask_sb = const.tile([128, ST], FP32, name="mask_sb")
        nc.sync.dma_start(out=mask_sb, in_=maskB[:, :])
        ones1 = const.tile([1, 128], FP32R, name="ones1")
        ones64 = const.tile([1, 64], FP32R, name="ones64")
        ones_sq = const.tile([128, 128], FP32R, name="ones_sq")
        nc.vector.memset(ones1[:].bitcast(FP32), 1.0)
        nc.vector.memset(ones64[:].bitcast(FP32), 1.0)
        nc.vector.memset(ones_sq[:].bitcast(FP32), 1.0)

        # persistent v tiles [128, 8 heads x 65] with ones columns at slot 64
        v_sb = [const.tile([128, H, 65], FP32R, tag=f"v{st}", name=f"v{st}")
                for st in range(ST)]
        for st in range(ST):
            nc.vector.memset(v_sb[st][:, :, 64:65].bitcast(FP32), 1.0)

        # ---- x = xT + peT  (feature-major, fp32r) ----
        x_r = [big.tile([128, S], FP32R, tag=f"x{dc}", name=f"x{dc}")
               for dc in range(DC)]
        for dc in range(DC):
            xt = tmp1k.tile([128, S], FP32, tag="t1a", name="ldx")
            pt = tmp1k.tile([128, S], FP32, tag="t1b", name="ldp")
            nc.sync.dma_start(out=xt, in_=xT[dc * 128:(dc + 1) * 128, :])
            nc.sync.dma_start(out=pt, in_=peT[dc * 128:(dc + 1) * 128, :])
            nc.vector.tensor_add(x_r[dc][:], xt[:], pt[:])

        def psA_halves(n):
            """yield n [128,1024] psum views, two per 4-bank psA slot"""
            views = []
            for i in range((n + 1) // 2):
                t = psA.tile([128, 2048], FP32, tag="psA", name="psA")
                views.append(t[:, 0:1024])
                if len(views) < n:
                    views.append(t[:, 1024:2048])
            return views

        def layernorm(zin, g_sb, be_sb, yout_dtype, yout_tags, dbg=""):
            """Feature-major LN over the partition dim (512 feats across 4 tiles)."""
            sums = psA.tile([128, 2048], FP32, tag="psA", name="sums")
            sum_z, sum_zz = sums[:, 0:1024], sums[:, 1024:2048]
            for sh in range(SH):
                sl = slice(sh * 512, (sh + 1) * 512)
                for kc in range(DC):
                    nc.tensor.matmul(sum_z[:, sl], ones_sq[:], zin[kc][:, sl],
                                     start=(kc == 0), stop=(kc == DC - 1))
            for sh in range(SH):
                sl = slice(sh * 512, (sh + 1) * 512)
                for kc in range(DC):
                    zz = tmp5.tile([128, 512], FP32R, tag="hz", name="zz", bufs=4)
                    nc.vector.tensor_mul(zz[:], zin[kc][:, sl], zin[kc][:, sl])
                    nc.tensor.matmul(sum_zz[:, sl], ones_sq[:], zz[:],
                                     start=(kc == 0), stop=(kc == DC - 1))
            if dbg == "sums":
                yout = [big.tile([128, S], yout_dtype, tag=yout_tags[dc],
                                 name=f"ln_{yout_tags[dc]}") for dc in range(DC)]
                nc.vector.tensor_scalar_mul(yout[0][:], sum_z[:], 1.0)
                nc.vector.tensor_scalar_mul(yout[1][:], sum_zz[:], 1.0)
                nc.vector.tensor_scalar_mul(yout[2][:], sum_z[:], 1.0)
                nc.vector.tensor_scalar_mul(yout[3][:], sum_zz[:], 1.0)
                return yout
            mean = stat.tile([128, S], FP32, tag="sm", name="mean")
            sa = stat.tile([128, S], FP32, tag="sa", name="sa")   # es2 -> var -> bt
            sb = stat.tile([128, S], FP32, tag="sb", name="sb")   # m2s -> std
            sr = stat.tile([128, S], FP32, tag="sr", name="rstd")
            # var*(D-1) = sum_zz - sum_z^2/D; 1/(D-1) folds into Sqrt scale
            nc.vector.tensor_scalar_mul(sb[:], sum_z[:], 1.0 / math.sqrt(D))
            nc.vector.tensor_mul(sb[:], sb[:], sb[:])             # sum_z^2/D
            nc.vector.tensor_sub(sa[:], sum_zz[:], sb[:])         # var*(D-1)
            nc.scalar.activation(sb[:], sa[:], AF.Sqrt,
                                 scale=1.0 / (D - 1))             # std
            nc.vector.tensor_scalar_mul(mean[:], sum_z[:], 1.0 / D)  # off-chain
            nc.vector.tensor_scalar_add(sb[:], sb[:], EPS)
            nc.vector.reciprocal(sr[:], sb[:])                    # rstd
            nc.vector.tensor_mul(sa[:], mean[:], sr[:])           # bt = mean*rstd
            if dbg == "stats":
                yout = [big.tile([128, S], yout_dtype, tag=yout_tags[dc],
                                 name=f"ln_{yout_tags[dc]}") for dc in range(DC)]
                for dc, src in enumerate((mean, sa, sb, sr)):
                    nc.vector.tensor_scalar_mul(yout[dc][:], src[:], 1.0)
                return yout
            yout = [big.tile([128, S], yout_dtype, tag=yout_tags[dc],
                             name=f"ln_{yout_tags[dc]}") for dc in range(DC)]
            for dc in range(DC):
                t1 = tmp1k.tile([128, S], FP32, tag="t1a", name="lnt1")
                nc.gpsimd.tensor_mul(t1[:], zin[dc][:], sr[:])
                t2 = tmp1k.tile([128, S], FP32, tag="t1b", name="lnt2")
                nc.vector.tensor_sub(t2[:], t1[:], sa[:])
                nc.vector.tensor_scalar(
                    yout[dc][:], t2[:], g_sb[:, dc:dc + 1], be_sb[:, dc:dc + 1],
                    op0=OP.mult, op1=OP.add)
            return yout


        def dbg_out(tiles):
            for dc in range(DC):
                od = tmp1k.tile([128, S], FP32, tag="t1a", name="dbg")
                nc.vector.tensor_copy(od[:], tiles[dc][:].bitcast(FP32))
                nc.sync.dma_start(out=outT[dc * 128:(dc + 1) * 128, :], in_=od[:])

        for l in range(n_layers):
            # ---- per-layer bias/gain tiles (one packed DMA) ----
            ba = bias.tile([128, 8 * DC + FC], FP32, tag="ball", name="ball")
            nc.sync.dma_start(out=ba, in_=ball[l, :, :])
            (bq_sb, bk_sb, bo_sb, b2_sb, g1_sb, be1_sb, g2_sb, be2_sb) = (
                ba[:, i * DC:(i + 1) * DC] for i in range(8))
            b1_sb = ba[:, 8 * DC:8 * DC + FC]
            bv_row = bias.tile([1, D], FP32R, tag="bv", name="bvr")
            nc.sync.dma_start(out=bv_row, in_=bv[l:l + 1, :])

            def load_w(drt):
                w = wpool.tile([128, DC, D], FP32R, tag="wqkv", name="wqkv")
                nc.sync.dma_start(
                    out=w, in_=drt[l].rearrange("(ko p) d -> p ko d", p=128))
                return [w[:, kc, :] for kc in range(DC)]

            def proj_fm(w_rows, b_sb, out_tags):
                out = [big.tile([128, S], FP32R, tag=out_tags[dc],
                                name=f"p_{out_tags[dc]}") for dc in range(DC)]
                pss = psA_halves(DC)
                for dc in range(DC):
                    ps = pss[dc]
                    for sh in range(SH):
                        sl = slice(sh * 512, (sh + 1) * 512)
                        for kc in range(DC):
                            nc.tensor.matmul(
                                ps[:, sl], w_rows[kc][:, dc * 128:(dc + 1) * 128],
                                x_r[kc][:, sl], start=(kc == 0), stop=(kc == DC - 1))
                    nc.vector.tensor_scalar_add(out[dc][:], ps[:], b_sb[:, dc:dc + 1])
                return out

            wq_rows = load_w(Wq)
            qT = proj_fm(wq_rows, bq_sb, [f"q{dc}" for dc in range(DC)])
            wk_rows = load_w(Wk)
            kT = proj_fm(wk_rows, bk_sb, [f"k{dc}" for dc in range(DC)])
            if debug_phase == "qkv_q":
                dbg_out(qT)
                break

            # v token-major into the augmented per-head layout
            wv_rows = load_w(Wv)
            for st in range(ST):
                ps = psB.tile([128, 512], FP32, tag="psB", name="psv")
                for kc in range(DC):
                    nc.tensor.matmul(ps[:], x_r[kc][:, st * 128:(st + 1) * 128],
                                     wv_rows[kc][:], start=(kc == 0), stop=False)
                nc.tensor.matmul(ps[:], ones1[:], bv_row[:], start=False, stop=True)
                nc.scalar.copy(
                    v_sb[st][:, :, 0:64],
                    ps[:].rearrange("p (h d) -> p h d", h=H))

            if debug_phase == "qkv":
                dbg_out(kT)
                break
            # ---- attention ----
            attnT = [big.tile([128, S], FP32R, tag=f"at{dc}", name=f"at{dc}")
                     for dc in range(DC)]
            patALL = psA.tile([65, 2048], FP32, tag="psA", name="patALL")
            for hp in range(H // 2):
                for qh in range(SH):
                    qsl = slice(qh * 512, (qh + 1) * 512)
                    it = hp * SH + qh
                    off = (it % 2) * 1024
                    pA = patALL[:, off:off + 512]
                    pB = patALL[:, off + 512:off + 1024]
                    for kc in range(ST):
                        sA = psB.tile([128, 512], FP32, tag="psB", name="sA")
                        sB = psB.tile([128, 512], FP32, tag="psB", name="sB")
                        ksl = slice(kc * 128, (kc + 1) * 128)
                        nc.tensor.matmul(sA[:], kT[hp][0:64, ksl], qT[hp][0:64, qsl],
                                         start=True, stop=True, tile_position=(0, 0))
                        nc.tensor.matmul(sB[:], kT[hp][64:128, ksl],
                                         qT[hp][64:128, qsl],
                                         start=True, stop=True, tile_position=(64, 0))
                        eA = expp.tile([128, 512], FP32R, tag="eA", name="eA")
                        eB = expp.tile([128, 512], FP32R, tag="eB", name="eB")
                        nc.scalar.activation(eA[:], sA[:], AF.Exp,
                                             bias=mask_sb[:, kc:kc + 1],
                                             scale=1.0 / math.sqrt(DH))
                        nc.scalar.activation(eB[:], sB[:], AF.Exp,
                                             bias=mask_sb[:, kc:kc + 1],
                                             scale=1.0 / math.sqrt(DH))
                        if debug_phase != "attn_sc":
                            nc.tensor.matmul(pA[:], v_sb[kc][:, 2 * hp, :], eA[:],
                                             start=(kc == 0), stop=(kc == ST - 1))
                            nc.tensor.matmul(pB[:], v_sb[kc][:, 2 * hp + 1, :], eB[:],
                                             start=(kc == 0), stop=(kc == ST - 1))
                        elif kc == 0:
                            nc.vector.tensor_copy(attnT[hp][:, qsl],
                                                  eA[:].bitcast(FP32))
                    if debug_phase == "attn_sc":
                        continue
                    if debug_phase == "attn_mm":
                        nc.vector.tensor_copy(attnT[hp][0:65, qsl][0:64, :],
                                              pA[0:64, :])
                        nc.vector.tensor_copy(attnT[hp][64:128, qsl][0:64, :],
                                              pB[0:64, :])
                        continue
                    for loc, p in ((0, pA), (1, pB)):
                        rows = slice(loc * 64, loc * 64 + 64)
                        nc.vector.tensor_copy(attnT[hp][rows, qsl], p[0:64, :])
                        rd = tmp5.tile([1, 512], FP32R, tag="rd", name="rd",
                                       bufs=3)
                        with nc.allow_low_precision(reason="softmax denom fp32r"):
                            nc.vector.reciprocal(rd[:], p[64:65, :])
                        rb = tmp5.tile([128, 512], FP32R, tag="rb", name="rb")
                        nc.gpsimd.partition_broadcast(rb[:], rd[:])
                        nc.gpsimd.tensor_mul(attnT[hp][rows, qsl],
                                             attnT[hp][rows, qsl],
                                             rb[rows, :])

            if debug_phase.startswith("attn"):
                dbg_out(attnT)
                break
            # ---- Wo projection + residual -> z1, LN1 -> y1 ----
            wo_rows = load_w(Wo)
            z1 = [big.tile([128, S], FP32R, tag=f"q{dc}", name=f"z{dc}")
                  for dc in range(DC)]
            pss = psA_halves(DC)
            for dc in range(DC):
                ps = pss[dc]
                for sh in range(SH):
                    sl = slice(sh * 512, (sh + 1) * 512)
                    for kc in range(DC):
                        nc.tensor.matmul(
                            ps[:, sl], wo_rows[kc][:, dc * 128:(dc + 1) * 128],
                            attnT[kc][:, sl], start=(kc == 0), stop=(kc == DC - 1))
                to = tmp1k.tile([128, S], FP32, tag="t1a", name="to")
                nc.scalar.activation(to[:], ps[:], AF.Identity,
                                     bias=bo_sb[:, dc:dc + 1])
                nc.gpsimd.tensor_add(z1[dc][:], to[:], x_r[dc][:])

            if debug_phase == "wo":
                dbg_out(z1)
                break
            y1 = layernorm(z1, g1_sb, be1_sb, FP32R,
                           [f"k{dc}" for dc in range(DC)],
                           dbg=debug_phase.replace("ln1_", "")
                           if debug_phase.startswith("ln1_") else "")
            if debug_phase.startswith("ln1"):
                dbg_out(y1)
                break

            # ---- FFN ----
            z2 = [big.tile([128, S], FP32R, tag=f"at{dc}", name=f"z2_{dc}")
                  for dc in range(DC)]
            for sh in range(SH):
                sl = slice(sh * 512, (sh + 1) * 512)
                pf2 = psA.tile([128, 2048], FP32, tag="psA", name="pf2")
                for fc in range(FC):
                    w1f = wstrm.tile([128, DC, 128], FP32R, tag="w1f", name="w1f")
                    nc.sync.dma_start(
                        out=w1f,
                        in_=W1[l].rearrange("(ko p) f -> p ko f", p=128)[
                            :, :, fc * 128:(fc + 1) * 128])
                    if fc % 2 == 0:
                        w2q = wstrm.tile([128, 2, D], FP32R, tag="w2q", name="w2q")
                        nc.sync.dma_start(
                            out=w2q,
                            in_=W2[l].rearrange("(fo p) d -> p fo d", p=128)[
                                :, fc:fc + 2, :])
                    w2f = w2q[:, fc % 2, :]
                    pf = psB.tile([128, 512], FP32, tag="psB", name="pf")
                    for kc in range(DC):
                        nc.tensor.matmul(pf[:], w1f[:, kc, :], y1[kc][:, sl],
                                         start=(kc == 0), stop=(kc == DC - 1))
                    h_r = tmp5.tile([128, 512], FP32R, tag="hz", name="h", bufs=4)
                    nc.scalar.activation(h_r[:], pf[:], AF.Relu,
                                         bias=b1_sb[:, fc:fc + 1])
                    for dc in range(DC):
                        nc.tensor.matmul(
                            pf2[:, dc * 512:(dc + 1) * 512],
                            w2f[:, dc * 128:(dc + 1) * 128], h_r[:],
                            start=(fc == 0), stop=(fc == FC - 1))
                for dc in range(DC):
                    tf = tmp5.tile([128, 512], FP32, tag="tf", name="tf")
                    nc.scalar.activation(
                        tf[:], pf2[:, dc * 512:(dc + 1) * 512],
                        AF.Identity, bias=b2_sb[:, dc:dc + 1])
                    nc.gpsimd.tensor_add(z2[dc][:, sl], tf[:], y1[dc][:, sl])

            last = (l == n_layers - 1)
            xnew = layernorm(z2, g2_sb, be2_sb,
                             FP32 if last else FP32R,
                             [f"x{dc}" for dc in range(DC)])
            if last:
                for dc in range(DC):
                    nc.sync.dma_start(out=outT[dc * 128:(dc + 1) * 128, :],
                                      in_=xnew[dc][:])
            else:
                x_r = xnew

    nc.compile()
    return nc


_cache: dict = {}
_exec_time_ns = None
_last_res = None


def kernel(**inputs) -> np.ndarray:
    x = np.asarray(inputs["x"], dtype=np.float32)
    lens = np.asarray(inputs["lens"])
    pe = _pe_table()

    if "nc" not in _cache:
        _cache["nc"] = build_nc(L)
    nc = _cache["nc"]

    shared = {
        "peT": np.ascontiguousarray(pe.T),
        "Wq": round_fp32r(inputs["Wq"]),
        "Wk": round_fp32r(inputs["Wk"]),
        "Wv": round_fp32r(inputs["Wv"]),
        "Wo": round_fp32r(inputs["Wo"]),
        "W1": round_fp32r(inputs["W1"]),
        "W2": round_fp32r(inputs["W2"]),
        "bv": round_fp32r(inputs["bv"]),
    }
    cols = [np.asarray(inputs[n], dtype=np.float32).reshape(L, DC, 128)
            .transpose(0, 2, 1)
            for n in ("bq", "bk", "bo", "b2", "g1", "be1", "g2", "be2")]
    cols.append(np.asarray(inputs["b1"], dtype=np.float32).reshape(L, FC, 128)
                .transpose(0, 2, 1))
    shared["ball"] = np.ascontiguousarray(np.concatenate(cols, axis=2))

    in_maps = []
    for c in range(NCORES):
        m = dict(shared)
        m["xT"] = np.ascontiguousarray(x[c].T)               # [D, S]
        mask = np.where(np.arange(S) < int(lens[c]), 0.0, NEG).astype(np.float32)
        m["maskB"] = np.ascontiguousarray(mask.reshape(ST, 128).T)  # [128, ST]
        in_maps.append(m)

    res = run_bass_kernel_spmd(nc, in_maps, core_ids=list(range(NCORES)))
    global _exec_time_ns, _last_res
    _last_res = res
    _exec_time_ns = res.exec_time_ns
    out = np.stack([res.results[c]["outT"].T for c in range(NCORES)])
    return out.astype(np.float32)

